# revision 1
# baseline (speedup 1.0000x reference)
"""Segment-mean + linear head kernel for TRN2 (8 NeuronCores, data parallel).

Reference computation (per batch row r):
    seg-mean of x[r] over tokens sharing word_id, gathered back per token,
    then linear head W,b:  logits[r,s,:] = mean_{s': wid[s']=wid[s]} x[r,s'] @ W.T + b

Key identity: the mean and the linear head commute, so
    logits[r,s,:] = Z[wid[s],:]  with  Z[g,:] = (sum_{s in g} y[s,:]) / max(cnt_g,1) + b,
    y = x @ W.T   ([S,15] -- tiny channel dim).
The segment scatter/gather is done with 0/1 indicator matmuls on the tensor
engine; indicators are generated on-chip with iota + is_equal compares.
Word ids are sorted per row, so each 128-wide segment chunk only touches a
few contiguous 128-token tiles; that schedule is computed on the host from
the actual ids (union across cores so the SPMD program is identical).
"""

import sys
from contextlib import ExitStack

import numpy as np

for _p in ("/opt/trn_rl_repo",):
    if _p not in sys.path:
        sys.path.insert(0, _p)

import concourse.bass as bass
import concourse.bacc as bacc
import concourse.tile as tile
from concourse import mybir
from concourse.bass_utils import run_bass_kernel_spmd

B, S, H, C = 16, 2048, 1024, 15
NW = 800
NCORES = 8
RPC = B // NCORES          # rows per core
T = S // 128               # 128-token tiles per row
NK = H // 128              # 128-wide h chunks
NCHUNK = (NW + 127) // 128 # 128-wide segment chunks

F32 = mybir.dt.float32
F32R = mybir.dt.float32r
BF16 = mybir.dt.bfloat16
I32 = mybir.dt.int32
EQ = mybir.AluOpType.is_equal
MULT = mybir.AluOpType.mult


def _schedule(word_ids):
    """chunks_t[lr][t]: sorted segment-chunk ids present in tile t of local row
    lr on ANY core; windows[lr][j]: sorted tiles where chunk j is active."""
    cid = (np.asarray(word_ids).astype(np.int64) // 128).reshape(B, T, 128)
    chunks_t = [[set() for _ in range(T)] for _ in range(RPC)]
    for core in range(NCORES):
        for lr in range(RPC):
            g = core * RPC + lr
            for t in range(T):
                for j in np.unique(cid[g, t]):
                    chunks_t[lr][t].add(int(j))
    chunks_t = [[sorted(s) for s in row] for row in chunks_t]
    windows = [
        [[t for t in range(T) if j in chunks_t[lr][t]] for j in range(NCHUNK)]
        for lr in range(RPC)
    ]
    return chunks_t, windows


def _build(chunks_t, windows):
    nc = bacc.Bacc("TRN2", target_bir_lowering=False, debug=False)
    x_d = nc.declare_dram_parameter("x", [RPC, S, H], BF16, isOutput=False)
    widr_d = nc.declare_dram_parameter("widr", [RPC, S], F32R, isOutput=False)
    widc_d = nc.declare_dram_parameter("widc", [RPC, 128, T], F32, isOutput=False)
    wt_d = nc.declare_dram_parameter("wt", [NK, 128, C], BF16, isOutput=False)
    b_d = nc.declare_dram_parameter("bias", [1, 16], F32R, isOutput=False)
    out_d = nc.declare_dram_parameter("out", [RPC, 128, T * C], F32, isOutput=True)

    with tile.TileContext(nc) as tc, ExitStack() as ctx:
        consts = ctx.enter_context(tc.tile_pool(name="consts", bufs=1))
        widp = ctx.enter_context(tc.tile_pool(name="widp", bufs=2))
        xpool = ctx.enter_context(tc.tile_pool(name="xpool", bufs=3))
        xtpool = ctx.enter_context(tc.tile_pool(name="xtpool", bufs=2))
        ytsb = ctx.enter_context(tc.tile_pool(name="ytsb", bufs=2))
        y1p = ctx.enter_context(tc.tile_pool(name="y1p", bufs=4))
        apool = ctx.enter_context(tc.tile_pool(name="apool", bufs=4))
        zpool = ctx.enter_context(tc.tile_pool(name="zpool", bufs=2))
        scp = ctx.enter_context(tc.tile_pool(name="scp", bufs=4))
        opool = ctx.enter_context(tc.tile_pool(name="opool", bufs=2))
        tpps = ctx.enter_context(tc.tile_pool(name="tpps", bufs=3, space="PSUM"))
        ypps = ctx.enter_context(tc.tile_pool(name="ypps", bufs=2, space="PSUM"))
        smps = ctx.enter_context(tc.tile_pool(name="smps", bufs=2, space="PSUM"))

        # --- constants ---
        iotag = consts.tile([128, NCHUNK, 128], F32, tag="iotag")
        nc.gpsimd.iota(iotag[:], [[128, NCHUNK], [1, 128]], channel_multiplier=0,
                       allow_small_or_imprecise_dtypes=True)
        pidx = consts.tile([128, NCHUNK], F32, tag="pidx")
        nc.gpsimd.iota(pidx[:], [[128, NCHUNK]], channel_multiplier=1,
                       allow_small_or_imprecise_dtypes=True)
        i0 = consts.tile([128, 128], F32, tag="i0")
        nc.gpsimd.iota(i0[:], [[1, 128]], channel_multiplier=0,
                       allow_small_or_imprecise_dtypes=True)
        p0 = consts.tile([128, 1], F32, tag="p0")
        nc.gpsimd.iota(p0[:], [[0, 1]], channel_multiplier=1,
                       allow_small_or_imprecise_dtypes=True)
        ident = consts.tile([128, 128], F32, tag="ident")
        nc.vector.tensor_scalar(ident[:], i0[:], p0[:], None, op0=EQ)
        ident_bf = consts.tile([128, 128], BF16, tag="identbf")
        nc.vector.tensor_scalar(ident_bf[:], i0[:], p0[:], None, op0=EQ)
        wt_sb = consts.tile([128, NK, C], BF16, tag="wt")
        nc.sync.dma_start(wt_sb[:], wt_d.rearrange("k h c -> h k c"))
        b_sb = consts.tile([1, 16], F32R, tag="bias")
        nc.sync.dma_start(b_sb[:], b_d[:])
        ones_col = consts.tile([1, 128], F32R, tag="ones")
        nc.vector.memset(ones_col[:].bitcast(F32), 1.0)
        b_bc = consts.tile([128, 16], BF16, tag="bbc")
        bb_ps = smps.tile([128, 16], F32, tag="sm")
        nc.tensor.matmul(bb_ps[:], ones_col[:], b_sb[:], start=True, stop=True)
        nc.any.tensor_copy(b_bc[:], bb_ps[:])

        for r in range(RPC):
            ct = chunks_t[r]
            win = windows[r]
            present = [j for j in range(NCHUNK) if win[j]]

            widr_sb = widp.tile([1, S], F32R, tag="widr")
            nc.sync.dma_start(widr_sb[:], widr_d[r : r + 1, :])
            widc_sb = widp.tile([128, T], F32, tag="widc")
            nc.sync.dma_start(widc_sb[:], widc_d[r])
            xr = x_d[r].rearrange("(t p) h -> p t h", p=128)
            wid_bc = widp.tile([128, S], F32, tag="widbc")
            for q in range(S // 512):
                wb_ps = tpps.tile([128, 512], F32, tag="tp")
                nc.tensor.matmul(
                    wb_ps[:],
                    ones_col[:],
                    widr_sb[0:1, 512 * q : 512 * q + 512],
                    start=True,
                    stop=True,
                )
                nc.any.tensor_copy(wid_bc[:, 512 * q : 512 * q + 512], wb_ps[:])

            sums_sb = zpool.tile([128, NCHUNK, 16], F32, tag="sums")
            nc.vector.memset(sums_sb[:], 0.0)
            # --- pass 1: y = x@W.T per token, scatter into segment sums ---
            for g4 in range(T // 4):
                x_sb = xpool.tile([128, 4, H], BF16)
                nc.sync.dma_start(x_sb[:], xr[:, 4 * g4 : 4 * g4 + 4, :])
                xt_sb = xtpool.tile([128, NK, 512], BF16)
                for ti in range(4):
                    for half in range(2):
                        tp = tpps.tile([128, 512], BF16, tag="tp")
                        for kk in range(4):
                            k = 4 * half + kk
                            nc.tensor.transpose(
                                tp[:, 128 * kk : 128 * kk + 128],
                                x_sb[:, ti, 128 * k : 128 * k + 128],
                                ident_bf[:],
                            )
                        nc.any.tensor_copy(
                            xt_sb[:, 4 * half : 4 * half + 4, 128 * ti : 128 * ti + 128],
                            tp[:].rearrange("p (k s) -> p k s", k=4),
                        )
                yp = ypps.tile([C, 512], F32)
                for k in range(NK):
                    nc.tensor.matmul(
                        yp[:],
                        wt_sb[:, k, :],
                        xt_sb[:, k, :],
                        start=(k == 0),
                        stop=(k == NK - 1),
                    )
                yt = ytsb.tile([C, 512], BF16)
                nc.any.tensor_copy(yt[:], yp[:])
                for ti in range(4):
                    t = 4 * g4 + ti
                    ytp = smps.tile([128, 16], BF16, tag="sm")
                    nc.tensor.transpose(
                        ytp[:, 0:C],
                        yt[:, 128 * ti : 128 * ti + 128],
                        ident_bf[:C, :C],
                    )
                    y1 = y1p.tile([128, 16], BF16)
                    nc.any.tensor_copy(y1[:, 0:C], ytp[:, 0:C])
                    nc.vector.memset(y1[:, C : C + 1], 1.0)
                    for j in ct[t]:
                        a = apool.tile([128, 128], BF16, tag="a")
                        nc.vector.tensor_scalar(
                            a[:], iotag[:, j, :], widc_sb[:, t : t + 1], None, op0=EQ
                        )
                        part = smps.tile([128, 16], F32, tag="sm")
                        nc.tensor.matmul(
                            part[:],
                            a[:],
                            y1[:],
                            start=True,
                            stop=True,
                        )
                        nc.vector.tensor_add(
                            sums_sb[:, j, :], sums_sb[:, j, :], part[:]
                        )

            # --- Z: means + bias per segment chunk ---
            z_sb = zpool.tile([128, NCHUNK, 16], BF16, tag="z")
            for j in present:
                cm = scp.tile([128, 1], F32, tag="cm")
                nc.vector.tensor_scalar_max(cm[:], sums_sb[:, j, C : C + 1], 1.0)
                rc = scp.tile([128, 1], F32, tag="rc")
                nc.vector.reciprocal(rc[:], cm[:])
                nc.vector.tensor_scalar(
                    z_sb[:, j, :], sums_sb[:, j, :], rc[:], None, op0=MULT
                )
                nc.vector.tensor_add(z_sb[:, j, :], z_sb[:, j, :], b_bc[:])

            # --- pass 2: gather Z back to tokens ---
            orow = opool.tile([128, T * C], F32)
            for t in range(T):
                ops_ = smps.tile([128, 16], F32, tag="sm")
                cl = ct[t]
                for idx, j in enumerate(cl):
                    at = apool.tile([128, 128], BF16, tag="a")
                    nc.vector.tensor_scalar(
                        at[:],
                        wid_bc[:, 128 * t : 128 * t + 128],
                        pidx[:, j : j + 1],
                        None,
                        op0=EQ,
                    )
                    nc.tensor.matmul(
                        ops_[:],
                        at[:],
                        z_sb[:, j, :],
                        start=(idx == 0),
                        stop=(idx == len(cl) - 1),
                    )
                nc.any.tensor_copy(orow[:, C * t : C * t + C], ops_[:, 0:C])
            nc.sync.dma_start(out_d[r], orow[:])

    nc.compile()
    return nc


def _prep_host(x, word_ids, W, b):
    import ml_dtypes
    wid32 = np.ascontiguousarray(np.asarray(word_ids).astype(np.int64))
    widf = wid32.astype(np.float32)
    widc = np.ascontiguousarray(
        widf.reshape(B, T, 128).transpose(0, 2, 1)
    )  # [B,128,T]
    wtk = np.ascontiguousarray(
        np.asarray(W, dtype=np.float32).T.reshape(NK, 128, C)
    ).astype(ml_dtypes.bfloat16)
    bp = np.zeros((1, 16), dtype=np.float32)
    bp[0, :C] = np.asarray(b, dtype=np.float32)
    return wid32, widf, widc, wtk, bp


def _run(x, word_ids, W, b, **spmd_kwargs):
    import ml_dtypes
    x = np.ascontiguousarray(np.asarray(x, dtype=np.float32)).astype(ml_dtypes.bfloat16)
    wid32, widf, widc, wtk, bp = _prep_host(x, word_ids, W, b)
    chunks_t, windows = _schedule(wid32)
    nc = _build(chunks_t, windows)

    in_maps = []
    for core in range(NCORES):
        r0 = core * RPC
        in_maps.append(
            {
                "x": x[r0 : r0 + RPC],
                "widr": widf[r0 : r0 + RPC],
                "widc": widc[r0 : r0 + RPC],
                "wt": wtk,
                "bias": bp,
            }
        )
    res = run_bass_kernel_spmd(nc, in_maps, list(range(NCORES)), **spmd_kwargs)
    outs = []
    for core in range(NCORES):
        o = res.results[core]["out"]  # [RPC, 128, T*C]
        o = o.reshape(RPC, 128, T, C).transpose(0, 2, 1, 3).reshape(RPC, S, C)
        outs.append(o)
    full = np.ascontiguousarray(np.concatenate(outs, axis=0).astype(np.float32))
    return full, res


def kernel(x, word_ids, W, b):
    return _run(x, word_ids, W, b)[0]


if __name__ == "__main__":
    rng = np.random.default_rng(0)
    x = rng.standard_normal((B, S, H), dtype=np.float32)
    wid = np.sort(rng.integers(0, NW, (B, S)), axis=-1)
    W = rng.standard_normal((C, H), dtype=np.float32) / np.sqrt(H)
    b = np.zeros((C,), dtype=np.float32)
    out = kernel(x, wid, W, b)
    print(out.shape, out.dtype)



# revision 3
# speedup vs baseline: 4.2870x; 4.2870x over previous
"""Segment-mean + linear head kernel for TRN2 (8 NeuronCores, data parallel).

Reference computation (per batch row r):
    seg-mean of x[r] over tokens sharing word_id, gathered back per token,
    then linear head W,b:  logits[r,s,:] = mean_{s': wid[s']=wid[s]} x[r,s'] @ W.T + b

Key identity: the mean and the linear head commute, so
    logits[r,s,:] = Z[wid[s],:]  with  Z[g,:] = (sum_{s in g} y[s,:]) * rc_g + b,
    y = x @ W.T   ([S,15] -- tiny channel dim), rc_g = 1/max(count_g,1).
The segment scatter/gather is done with 0/1 indicator matmuls on the tensor
engine; indicators are generated on-chip with iota + is_equal compares.
Word ids are sorted per row, so each 128-wide segment chunk is only active in
a contiguous window of 128-token tiles; the scatter accumulates directly in
PSUM across that window. The schedule is computed on the host from the actual
ids (union across cores so the SPMD program is identical on every core).

Host-side prep keeps the device program small: x is shipped pre-transposed
([B,NK,128,S] bf16) so no on-chip transposes of x are needed, and the
per-segment reciprocal counts are computed on the host (f32) so the device
does a single multiply per segment chunk instead of max+reciprocal.

The runner compiles the Bass program once per segment schedule (memoized) and
starts the device upload of the inputs before dispatch so the transfer
overlaps any remaining host work.
"""

import sys
from contextlib import ExitStack

import numpy as np

for _p in ("/opt/trn_rl_repo",):
    if _p not in sys.path:
        sys.path.insert(0, _p)

import concourse.bass as bass
import concourse.bacc as bacc
import concourse.tile as tile
from concourse import mybir

B, S, H, C = 16, 2048, 1024, 15
NW = 800
NCORES = 8
RPC = B // NCORES          # rows per core
T = S // 128               # 128-token tiles per row
NK = H // 128              # 128-wide h chunks
NCHUNK = (NW + 127) // 128 # 128-wide segment chunks

F32 = mybir.dt.float32
F32R = mybir.dt.float32r
BF16 = mybir.dt.bfloat16
EQ = mybir.AluOpType.is_equal
MULT = mybir.AluOpType.mult


def _schedule(wid64):
    """chunks_t[lr][t]: segment-chunk ids spanned by tile t of local row lr on
    ANY core (ids are sorted per row, so a tile spans a contiguous chunk
    range); first/last[lr][j]: tile window in which chunk j is active."""
    cid = (wid64 // 128).reshape(B, T, 128)
    chunks_t = []
    for lr in range(RPC):
        row = []
        for t in range(T):
            lo = min(int(cid[core * RPC + lr, t, 0]) for core in range(NCORES))
            hi = max(int(cid[core * RPC + lr, t, -1]) for core in range(NCORES))
            row.append(tuple(range(lo, hi + 1)))
        chunks_t.append(tuple(row))
    first, last, overlap = [], [], 2
    for lr in range(RPC):
        f = {}
        l = {}
        for t in range(T):
            for j in chunks_t[lr][t]:
                f.setdefault(j, t)
                l[j] = t
        first.append(f)
        last.append(l)
        for t in range(T):
            overlap = max(overlap, sum(1 for j in f if f[j] <= t <= l[j]))
    return tuple(chunks_t), first, last, overlap


def _build(chunks_t, first, last, sc_bufs):
    nc = bacc.Bacc("TRN2", target_bir_lowering=False, debug=False)
    xt_d = nc.declare_dram_parameter("xt", [RPC, NK, 128, S], BF16, isOutput=False)
    widr_d = nc.declare_dram_parameter("widr", [RPC, S], F32R, isOutput=False)
    widc_d = nc.declare_dram_parameter("widc", [RPC, 128, T], F32, isOutput=False)
    rc_d = nc.declare_dram_parameter("rcc", [RPC, 128, NCHUNK], F32, isOutput=False)
    wt_d = nc.declare_dram_parameter("wt", [NK, 128, C], BF16, isOutput=False)
    b_d = nc.declare_dram_parameter("bias", [128, C], F32, isOutput=False)
    out_d = nc.declare_dram_parameter("out", [RPC, 128, T * C], F32, isOutput=True)

    # PSUM budget is 8 banks; tp/yp/sm take 6, so the scatter accumulator
    # pool can hold at most 2 concurrently-open windows at the default.
    tp_bufs = 2 if sc_bufs <= 2 else 1
    yp_bufs = 2 if sc_bufs <= 3 else 1

    with tile.TileContext(nc) as tc, ExitStack() as ctx:
        consts = ctx.enter_context(tc.tile_pool(name="consts", bufs=1))
        widp = ctx.enter_context(tc.tile_pool(name="widp", bufs=2))
        xpool = ctx.enter_context(tc.tile_pool(name="xpool", bufs=2))
        ytsb = ctx.enter_context(tc.tile_pool(name="ytsb", bufs=2))
        y1p = ctx.enter_context(tc.tile_pool(name="y1p", bufs=4))
        apool = ctx.enter_context(tc.tile_pool(name="apool", bufs=4))
        zpool = ctx.enter_context(tc.tile_pool(name="zpool", bufs=2))
        opool = ctx.enter_context(tc.tile_pool(name="opool", bufs=2))
        tpps = ctx.enter_context(tc.tile_pool(name="tpps", bufs=tp_bufs, space="PSUM"))
        ypps = ctx.enter_context(tc.tile_pool(name="ypps", bufs=yp_bufs, space="PSUM"))
        smps = ctx.enter_context(tc.tile_pool(name="smps", bufs=2, space="PSUM"))
        scps = ctx.enter_context(tc.tile_pool(name="scps", bufs=sc_bufs, space="PSUM"))

        # --- constants ---
        iotag = consts.tile([128, NCHUNK, 128], F32, tag="iotag")
        nc.gpsimd.iota(iotag[:], [[128, NCHUNK], [1, 128]], channel_multiplier=0,
                       allow_small_or_imprecise_dtypes=True)
        pidx = consts.tile([128, NCHUNK], F32, tag="pidx")
        nc.gpsimd.iota(pidx[:], [[128, NCHUNK]], channel_multiplier=1,
                       allow_small_or_imprecise_dtypes=True)
        i0 = consts.tile([128, 128], F32, tag="i0")
        nc.gpsimd.iota(i0[:], [[1, 128]], channel_multiplier=0,
                       allow_small_or_imprecise_dtypes=True)
        p0 = consts.tile([128, 1], F32, tag="p0")
        nc.gpsimd.iota(p0[:], [[0, 1]], channel_multiplier=1,
                       allow_small_or_imprecise_dtypes=True)
        ident_bf = consts.tile([128, 128], BF16, tag="identbf")
        nc.vector.tensor_scalar(ident_bf[:], i0[:], p0[:], None, op0=EQ)
        ones_col = consts.tile([1, 128], F32R, tag="ones")
        nc.vector.memset(ones_col[:].bitcast(F32), 1.0)
        wt_sb = consts.tile([128, NK, C], BF16, tag="wt")
        nc.sync.dma_start(wt_sb[:], wt_d.rearrange("k h c -> h k c"))
        b_bc = consts.tile([128, C], F32, tag="bias")
        nc.sync.dma_start(b_bc[:], b_d[:])

        for r in range(RPC):
            ct = chunks_t[r]
            fj, lj = first[r], last[r]

            widr_sb = widp.tile([1, S], F32R, tag="widr")
            nc.sync.dma_start(widr_sb[:], widr_d[r : r + 1, :])
            widc_sb = widp.tile([128, T], F32, tag="widc")
            nc.sync.dma_start(widc_sb[:], widc_d[r])
            rc_sb = widp.tile([128, NCHUNK], F32, tag="rc")
            nc.sync.dma_start(rc_sb[:], rc_d[r])
            xt_sb = xpool.tile([128, NK, S], BF16, tag="xt")
            nc.sync.dma_start(xt_sb[:], xt_d[r].rearrange("k p s -> p k s"))

            # broadcast word ids across partitions (for the gather compares)
            wid_bc = widp.tile([128, S], F32, tag="widbc")
            for q in range(S // 512):
                wb_ps = tpps.tile([128, 512], F32, tag="tp")
                nc.tensor.matmul(
                    wb_ps[:],
                    ones_col[:],
                    widr_sb[0:1, 512 * q : 512 * q + 512],
                    start=True,
                    stop=True,
                )
                nc.any.tensor_copy(wid_bc[:, 512 * q : 512 * q + 512], wb_ps[:])

            z_sb = zpool.tile([128, NCHUNK, C], BF16, tag="z")
            open_sc = {}
            # --- pass 1: y = x@W.T per token, scatter-accumulate segment sums
            #     in PSUM across each chunk's contiguous tile window ---
            for g in range(T // 4):
                yp = ypps.tile([C, 512], F32, tag="yp")
                for k in range(NK):
                    nc.tensor.matmul(
                        yp[:],
                        wt_sb[:, k, :],
                        xt_sb[:, k, 512 * g : 512 * g + 512],
                        start=(k == 0),
                        stop=(k == NK - 1),
                    )
                yt = ytsb.tile([C, 512], BF16, tag="yt")
                nc.any.tensor_copy(yt[:], yp[:])
                for ti in range(4):
                    t = 4 * g + ti
                    ytp = smps.tile([128, 16], BF16, tag="sm")
                    nc.tensor.transpose(
                        ytp[:, 0:C],
                        yt[:, 128 * ti : 128 * ti + 128],
                        ident_bf[:C, :C],
                    )
                    y1 = y1p.tile([128, C], BF16, tag="y1")
                    nc.any.tensor_copy(y1[:], ytp[:, 0:C])
                    for j in ct[t]:
                        a = apool.tile([128, 128], BF16, tag="a")
                        nc.vector.tensor_scalar(
                            a[:], iotag[:, j, :], widc_sb[:, t : t + 1], None, op0=EQ
                        )
                        if t == fj[j]:
                            open_sc[j] = scps.tile(
                                [128, C], F32, tag="sc", name=f"sc_r{r}_j{j}"
                            )
                        nc.tensor.matmul(
                            open_sc[j][:],
                            a[:],
                            y1[:],
                            start=(t == fj[j]),
                            stop=(t == lj[j]),
                        )
                        if t == lj[j]:
                            # finalize chunk j: mean (host-side reciprocal
                            # counts) + bias
                            nc.vector.tensor_scalar(
                                z_sb[:, j, :],
                                open_sc[j][:],
                                rc_sb[:, j : j + 1],
                                None,
                                op0=MULT,
                            )
                            nc.vector.tensor_add(z_sb[:, j, :], z_sb[:, j, :], b_bc[:])
                            del open_sc[j]

            # --- pass 2: gather Z back to tokens ---
            orow = opool.tile([128, T * C], F32, tag="orow")
            for t in range(T):
                ops_ = smps.tile([128, 16], F32, tag="sm")
                cl = ct[t]
                for idx, j in enumerate(cl):
                    at = apool.tile([128, 128], BF16, tag="a")
                    nc.vector.tensor_scalar(
                        at[:],
                        wid_bc[:, 128 * t : 128 * t + 128],
                        pidx[:, j : j + 1],
                        None,
                        op0=EQ,
                    )
                    nc.tensor.matmul(
                        ops_[:, 0:C],
                        at[:],
                        z_sb[:, j, :],
                        start=(idx == 0),
                        stop=(idx == len(cl) - 1),
                    )
                nc.any.tensor_copy(orow[:, C * t : C * t + C], ops_[:, 0:C])
            nc.sync.dma_start(out_d[r], orow[:])

    nc.compile()
    return nc


def _prep_host(x, word_ids, W, b):
    import ml_dtypes

    wid64 = np.asarray(word_ids).astype(np.int64)
    xt = (
        np.asarray(x, dtype=np.float32)
        .reshape(B, S, NK, 128)
        .transpose(0, 2, 3, 1)
        .astype(ml_dtypes.bfloat16)
    )  # [B, NK, 128, S]
    widf = wid64.astype(np.float32)
    widc = np.ascontiguousarray(widf.reshape(B, T, 128).transpose(0, 2, 1))  # [B,128,T]
    seg = (wid64 + NW * np.arange(B, dtype=np.int64)[:, None]).reshape(-1)
    counts = np.bincount(seg, minlength=B * NW).reshape(B, NW)
    rc = np.zeros((B, NCHUNK * 128), dtype=np.float32)
    rc[:, :NW] = 1.0 / np.maximum(counts, 1)
    rcc = np.ascontiguousarray(
        rc.reshape(B, NCHUNK, 128).transpose(0, 2, 1)
    )  # [B,128,NCHUNK]
    wtk = np.ascontiguousarray(
        np.asarray(W, dtype=np.float32).T.reshape(NK, 128, C)
    ).astype(ml_dtypes.bfloat16)
    bias_bc = np.ascontiguousarray(
        np.broadcast_to(np.asarray(b, dtype=np.float32), (128, C))
    )
    return wid64, xt, widf, widc, rcc, wtk, bias_bc


_CACHE: dict = {}


def _get_compiled(chunks_t, first, last, overlap):
    entry = _CACHE.get(chunks_t)
    if entry is not None:
        return entry

    import jax
    from jax.experimental.shard_map import shard_map
    from jax.sharding import Mesh, NamedSharding, PartitionSpec
    from concourse.bass2jax import _bass_exec_p, install_neuronx_cc_hook

    install_neuronx_cc_hook()
    nc = _build(chunks_t, first, last, max(2, overlap))

    in_names: list[str] = []
    out_names: list[str] = []
    out_avals = []
    for alloc in nc.m.functions[0].allocations:
        if not isinstance(alloc, mybir.MemoryLocationSet):
            continue
        name = alloc.memorylocations[0].name
        if alloc.kind == "ExternalInput":
            in_names.append(name)
        elif alloc.kind == "ExternalOutput":
            out_names.append(name)
            out_avals.append(
                jax.core.ShapedArray(
                    tuple(alloc.tensor_shape), mybir.dt.np(alloc.dtype)
                )
            )
    n_params = len(in_names)
    n_outs = len(out_names)
    all_names = tuple(in_names + out_names)
    donate = tuple(range(n_params, n_params + n_outs))

    def _body(*args):
        outs = _bass_exec_p.bind(
            *args,
            out_avals=tuple(out_avals),
            in_names=all_names,
            out_names=tuple(out_names),
            lowering_input_output_aliases=(),
            sim_require_finite=True,
            sim_require_nnan=True,
            nc=nc,
        )
        return tuple(outs)

    devices = jax.devices()[:NCORES]
    mesh = Mesh(np.asarray(devices), ("core",))
    spec = PartitionSpec("core")
    fn = jax.jit(
        shard_map(
            _body,
            mesh=mesh,
            in_specs=(spec,) * (n_params + n_outs),
            out_specs=(spec,) * n_outs,
            check_rep=False,
        ),
        donate_argnums=donate,
        keep_unused=True,
    )
    entry = {
        "fn": fn,
        "in_names": tuple(in_names),
        "out_names": tuple(out_names),
        "sharding": NamedSharding(mesh, spec),
        "nc": nc,
    }
    _CACHE[chunks_t] = entry
    return entry


def _run_fast(x, word_ids, W, b):
    import jax

    wid64, xt, widf, widc, rcc, wtk, bias_bc = _prep_host(x, word_ids, W, b)
    # Globals along axis 0: per-core shard = rows [2*core, 2*core+2); the
    # replicated head weights are tiled 8x (tiny).
    globals_np = {
        "xt": xt,
        "widr": widf,
        "widc": widc,
        "rcc": rcc,
        "wt": np.tile(wtk, (NCORES, 1, 1)),
        "bias": np.tile(bias_bc, (NCORES, 1)),
    }
    zero_out = np.zeros((B, 128, T * C), dtype=np.float32)

    chunks_t, first, last, overlap = _schedule(wid64)
    entry = _CACHE.get(chunks_t)
    if entry is not None:
        # Warm path: start the async upload before dispatch so the transfer
        # overlaps host-side arg marshalling.
        sh = entry["sharding"]
        names = list(globals_np)
        devs = jax.device_put(
            [globals_np[n] for n in names] + [zero_out], [sh] * (len(names) + 1)
        )
        dev_map = dict(zip(names, devs[:-1]))
        out_devs = devs[-1:]
    else:
        entry = _get_compiled(chunks_t, first, last, overlap)
        sh = entry["sharding"]
        names = list(globals_np)
        devs = jax.device_put(
            [globals_np[n] for n in names] + [zero_out], [sh] * (len(names) + 1)
        )
        dev_map = dict(zip(names, devs[:-1]))
        out_devs = devs[-1:]

    args = [dev_map[n] for n in entry["in_names"]] + out_devs
    outs = entry["fn"](*args)
    out = np.asarray(outs[0])  # [B, 128, T*C]
    return (
        np.ascontiguousarray(
            out.reshape(B, 128, T, C).transpose(0, 2, 1, 3).reshape(B, S, C)
        ),
        None,
    )


def _run_fallback(x, word_ids, W, b, **spmd_kwargs):
    from concourse.bass_utils import run_bass_kernel_spmd

    wid64, xt, widf, widc, rcc, wtk, bias_bc = _prep_host(x, word_ids, W, b)
    chunks_t, first, last, overlap = _schedule(wid64)
    nc = _build(chunks_t, first, last, max(2, overlap))
    in_maps = []
    for core in range(NCORES):
        r0 = core * RPC
        in_maps.append(
            {
                "xt": xt[r0 : r0 + RPC],
                "widr": widf[r0 : r0 + RPC],
                "widc": widc[r0 : r0 + RPC],
                "rcc": rcc[r0 : r0 + RPC],
                "wt": wtk,
                "bias": bias_bc,
            }
        )
    res = run_bass_kernel_spmd(nc, in_maps, list(range(NCORES)), **spmd_kwargs)
    outs = []
    for core in range(NCORES):
        o = res.results[core]["out"]  # [RPC, 128, T*C]
        outs.append(o.reshape(RPC, 128, T, C).transpose(0, 2, 1, 3).reshape(RPC, S, C))
    return np.ascontiguousarray(np.concatenate(outs, axis=0).astype(np.float32)), res


def _run(x, word_ids, W, b, **spmd_kwargs):
    if spmd_kwargs.get("trace"):
        return _run_fallback(x, word_ids, W, b, **spmd_kwargs)
    try:
        return _run_fast(x, word_ids, W, b)
    except Exception:
        import traceback

        traceback.print_exc()
        return _run_fallback(x, word_ids, W, b)


def kernel(x, word_ids, W, b):
    return _run(x, word_ids, W, b)[0]


if __name__ == "__main__":
    rng = np.random.default_rng(0)
    x = rng.standard_normal((B, S, H), dtype=np.float32)
    wid = np.sort(rng.integers(0, NW, (B, S)), axis=-1)
    W = rng.standard_normal((C, H), dtype=np.float32) / np.sqrt(H)
    b = np.zeros((C,), dtype=np.float32)
    out = kernel(x, wid, W, b)
    print(out.shape, out.dtype)


# revision 4
# speedup vs baseline: 12.9402x; 3.0185x over previous
"""Segment-mean + linear head kernel for TRN2 (8 NeuronCores, data parallel).

Reference computation (per batch row r):
    seg-mean of x[r] over tokens sharing word_id, gathered back per token,
    then linear head W,b:  logits[r,s,:] = mean_{s': wid[s']=wid[s]} x[r,s'] @ W.T + b

Key identity: the mean and the linear head commute, so
    logits[r,s,:] = Z[wid[s],:]  with  Z[g,:] = (sum_{s in g} y[s,:]) * rc_g + b,
    y = x @ W.T   ([S,15] -- tiny channel dim), rc_g = 1/max(count_g,1).
The segment scatter/gather is done with 0/1 indicator matmuls on the tensor
engine; indicators are generated on-chip with iota + is_equal compares.
Word ids are sorted per row, so each 128-wide segment chunk is only active in
a contiguous window of 128-token tiles; the scatter accumulates directly in
PSUM across that window. The schedule is computed on the host from the actual
ids (union across cores so the SPMD program is identical on every core).

Host-side prep keeps the device program small: x is shipped pre-transposed
([B,NK,128,S] bf16) so no on-chip transposes of x are needed, and the
per-segment reciprocal counts are computed on the host (f32) so the device
does a single multiply per segment chunk instead of max+reciprocal.

The runner compiles the Bass program once per segment schedule (memoized) and
starts the device upload of the inputs before dispatch so the transfer
overlaps any remaining host work.
"""

import sys
from contextlib import ExitStack

import numpy as np

for _p in ("/opt/trn_rl_repo",):
    if _p not in sys.path:
        sys.path.insert(0, _p)

import concourse.bass as bass
import concourse.bacc as bacc
import concourse.tile as tile
from concourse import mybir

B, S, H, C = 16, 2048, 1024, 15
NW = 800
NCORES = 8
RPC = B // NCORES          # rows per core
T = S // 128               # 128-token tiles per row
NK = H // 128              # 128-wide h chunks
NCHUNK = (NW + 127) // 128 # 128-wide segment chunks

F32 = mybir.dt.float32
F32R = mybir.dt.float32r
BF16 = mybir.dt.bfloat16
EQ = mybir.AluOpType.is_equal
MULT = mybir.AluOpType.mult


def _schedule(wid64):
    """chunks_t[lr][t]: segment-chunk ids spanned by tile t of local row lr on
    ANY core (ids are sorted per row, so a tile spans a contiguous chunk
    range); first/last[lr][j]: tile window in which chunk j is active."""
    cid = (wid64 // 128).reshape(B, T, 128)
    chunks_t = []
    for lr in range(RPC):
        row = []
        for t in range(T):
            lo = min(int(cid[core * RPC + lr, t, 0]) for core in range(NCORES))
            hi = max(int(cid[core * RPC + lr, t, -1]) for core in range(NCORES))
            row.append(tuple(range(lo, hi + 1)))
        chunks_t.append(tuple(row))
    first, last, overlap = [], [], 2
    for lr in range(RPC):
        f = {}
        l = {}
        for t in range(T):
            for j in chunks_t[lr][t]:
                f.setdefault(j, t)
                l[j] = t
        first.append(f)
        last.append(l)
        for t in range(T):
            overlap = max(overlap, sum(1 for j in f if f[j] <= t <= l[j]))
    return tuple(chunks_t), first, last, overlap


def _build(chunks_t, first, last, sc_bufs):
    nc = bacc.Bacc("TRN2", target_bir_lowering=False, debug=False)
    xt_d = nc.declare_dram_parameter("xt", [RPC, NK, 128, S], BF16, isOutput=False)
    widr_d = nc.declare_dram_parameter("widr", [RPC, S], F32R, isOutput=False)
    widc_d = nc.declare_dram_parameter("widc", [RPC, 128, T], F32, isOutput=False)
    rc_d = nc.declare_dram_parameter("rcc", [RPC, 128, NCHUNK], F32, isOutput=False)
    wt_d = nc.declare_dram_parameter("wt", [NK, 128, C], BF16, isOutput=False)
    b_d = nc.declare_dram_parameter("bias", [128, C], F32, isOutput=False)
    out_d = nc.declare_dram_parameter("out", [RPC, 128, T * C], F32, isOutput=True)

    # PSUM budget is 8 banks; tp/yp/sm take 6, so the scatter accumulator
    # pool can hold at most 2 concurrently-open windows at the default.
    tp_bufs = 2 if sc_bufs <= 2 else 1
    yp_bufs = 2 if sc_bufs <= 3 else 1

    with tile.TileContext(nc) as tc, ExitStack() as ctx:
        consts = ctx.enter_context(tc.tile_pool(name="consts", bufs=1))
        widp = ctx.enter_context(tc.tile_pool(name="widp", bufs=2))
        xpool = ctx.enter_context(tc.tile_pool(name="xpool", bufs=2))
        ytsb = ctx.enter_context(tc.tile_pool(name="ytsb", bufs=2))
        y1p = ctx.enter_context(tc.tile_pool(name="y1p", bufs=4))
        apool = ctx.enter_context(tc.tile_pool(name="apool", bufs=4))
        zpool = ctx.enter_context(tc.tile_pool(name="zpool", bufs=2))
        opool = ctx.enter_context(tc.tile_pool(name="opool", bufs=2))
        tpps = ctx.enter_context(tc.tile_pool(name="tpps", bufs=tp_bufs, space="PSUM"))
        ypps = ctx.enter_context(tc.tile_pool(name="ypps", bufs=yp_bufs, space="PSUM"))
        smps = ctx.enter_context(tc.tile_pool(name="smps", bufs=2, space="PSUM"))
        scps = ctx.enter_context(tc.tile_pool(name="scps", bufs=sc_bufs, space="PSUM"))

        # --- constants ---
        iotag = consts.tile([128, NCHUNK, 128], F32, tag="iotag")
        nc.gpsimd.iota(iotag[:], [[128, NCHUNK], [1, 128]], channel_multiplier=0,
                       allow_small_or_imprecise_dtypes=True)
        pidx = consts.tile([128, NCHUNK], F32, tag="pidx")
        nc.gpsimd.iota(pidx[:], [[128, NCHUNK]], channel_multiplier=1,
                       allow_small_or_imprecise_dtypes=True)
        i0 = consts.tile([128, 128], F32, tag="i0")
        nc.gpsimd.iota(i0[:], [[1, 128]], channel_multiplier=0,
                       allow_small_or_imprecise_dtypes=True)
        p0 = consts.tile([128, 1], F32, tag="p0")
        nc.gpsimd.iota(p0[:], [[0, 1]], channel_multiplier=1,
                       allow_small_or_imprecise_dtypes=True)
        ident_bf = consts.tile([128, 128], BF16, tag="identbf")
        nc.vector.tensor_scalar(ident_bf[:], i0[:], p0[:], None, op0=EQ)
        ones_col = consts.tile([1, 128], F32R, tag="ones")
        nc.vector.memset(ones_col[:].bitcast(F32), 1.0)
        wt_sb = consts.tile([128, NK, C], BF16, tag="wt")
        nc.sync.dma_start(wt_sb[:], wt_d.rearrange("k h c -> h k c"))
        b_bc = consts.tile([128, C], F32, tag="bias")
        nc.sync.dma_start(b_bc[:], b_d[:])

        for r in range(RPC):
            ct = chunks_t[r]
            fj, lj = first[r], last[r]

            widr_sb = widp.tile([1, S], F32R, tag="widr")
            nc.sync.dma_start(widr_sb[:], widr_d[r : r + 1, :])
            widc_sb = widp.tile([128, T], F32, tag="widc")
            nc.sync.dma_start(widc_sb[:], widc_d[r])
            rc_sb = widp.tile([128, NCHUNK], F32, tag="rc")
            nc.sync.dma_start(rc_sb[:], rc_d[r])
            xt_sb = xpool.tile([128, NK, S], BF16, tag="xt")
            nc.sync.dma_start(xt_sb[:], xt_d[r].rearrange("k p s -> p k s"))

            # broadcast word ids across partitions (for the gather compares)
            wid_bc = widp.tile([128, S], F32, tag="widbc")
            for q in range(S // 512):
                wb_ps = tpps.tile([128, 512], F32, tag="tp")
                nc.tensor.matmul(
                    wb_ps[:],
                    ones_col[:],
                    widr_sb[0:1, 512 * q : 512 * q + 512],
                    start=True,
                    stop=True,
                )
                nc.any.tensor_copy(wid_bc[:, 512 * q : 512 * q + 512], wb_ps[:])

            z_sb = zpool.tile([128, NCHUNK, C], BF16, tag="z")
            open_sc = {}
            # --- pass 1: y = x@W.T per token, scatter-accumulate segment sums
            #     in PSUM across each chunk's contiguous tile window ---
            for g in range(T // 4):
                yp = ypps.tile([C, 512], F32, tag="yp")
                for k in range(NK):
                    nc.tensor.matmul(
                        yp[:],
                        wt_sb[:, k, :],
                        xt_sb[:, k, 512 * g : 512 * g + 512],
                        start=(k == 0),
                        stop=(k == NK - 1),
                    )
                yt = ytsb.tile([C, 512], BF16, tag="yt")
                nc.any.tensor_copy(yt[:], yp[:])
                for ti in range(4):
                    t = 4 * g + ti
                    ytp = smps.tile([128, 16], BF16, tag="sm")
                    nc.tensor.transpose(
                        ytp[:, 0:C],
                        yt[:, 128 * ti : 128 * ti + 128],
                        ident_bf[:C, :C],
                    )
                    y1 = y1p.tile([128, C], BF16, tag="y1")
                    nc.any.tensor_copy(y1[:], ytp[:, 0:C])
                    for j in ct[t]:
                        a = apool.tile([128, 128], BF16, tag="a")
                        nc.vector.tensor_scalar(
                            a[:], iotag[:, j, :], widc_sb[:, t : t + 1], None, op0=EQ
                        )
                        if t == fj[j]:
                            open_sc[j] = scps.tile(
                                [128, C], F32, tag="sc", name=f"sc_r{r}_j{j}"
                            )
                        nc.tensor.matmul(
                            open_sc[j][:],
                            a[:],
                            y1[:],
                            start=(t == fj[j]),
                            stop=(t == lj[j]),
                        )
                        if t == lj[j]:
                            # finalize chunk j: mean (host-side reciprocal
                            # counts) + bias
                            nc.vector.tensor_scalar(
                                z_sb[:, j, :],
                                open_sc[j][:],
                                rc_sb[:, j : j + 1],
                                None,
                                op0=MULT,
                            )
                            nc.vector.tensor_add(z_sb[:, j, :], z_sb[:, j, :], b_bc[:])
                            del open_sc[j]

            # --- pass 2: gather Z back to tokens ---
            orow = opool.tile([128, T * C], F32, tag="orow")
            for t in range(T):
                ops_ = smps.tile([128, 16], F32, tag="sm")
                cl = ct[t]
                for idx, j in enumerate(cl):
                    at = apool.tile([128, 128], BF16, tag="a")
                    nc.vector.tensor_scalar(
                        at[:],
                        wid_bc[:, 128 * t : 128 * t + 128],
                        pidx[:, j : j + 1],
                        None,
                        op0=EQ,
                    )
                    nc.tensor.matmul(
                        ops_[:, 0:C],
                        at[:],
                        z_sb[:, j, :],
                        start=(idx == 0),
                        stop=(idx == len(cl) - 1),
                    )
                nc.any.tensor_copy(orow[:, C * t : C * t + C], ops_[:, 0:C])
            nc.sync.dma_start(out_d[r], orow[:])

    nc.compile()
    return nc


def _prep_host(x, word_ids, W, b):
    import ml_dtypes

    wid64 = np.asarray(word_ids).astype(np.int64)
    xt = (
        np.asarray(x, dtype=np.float32)
        .reshape(B, S, NK, 128)
        .transpose(0, 2, 3, 1)
        .astype(ml_dtypes.bfloat16)
    )  # [B, NK, 128, S]
    widf = wid64.astype(np.float32)
    widc = np.ascontiguousarray(widf.reshape(B, T, 128).transpose(0, 2, 1))  # [B,128,T]
    seg = (wid64 + NW * np.arange(B, dtype=np.int64)[:, None]).reshape(-1)
    counts = np.bincount(seg, minlength=B * NW).reshape(B, NW)
    rc = np.zeros((B, NCHUNK * 128), dtype=np.float32)
    rc[:, :NW] = 1.0 / np.maximum(counts, 1)
    rcc = np.ascontiguousarray(
        rc.reshape(B, NCHUNK, 128).transpose(0, 2, 1)
    )  # [B,128,NCHUNK]
    wtk = np.ascontiguousarray(
        np.asarray(W, dtype=np.float32).T.reshape(NK, 128, C)
    ).astype(ml_dtypes.bfloat16)
    bias_bc = np.ascontiguousarray(
        np.broadcast_to(np.asarray(b, dtype=np.float32), (128, C))
    )
    return wid64, xt, widf, widc, rcc, wtk, bias_bc


_CACHE: dict = {}


def _get_compiled(chunks_t, first, last, overlap):
    entry = _CACHE.get(chunks_t)
    if entry is not None:
        return entry

    import jax
    from jax.experimental.shard_map import shard_map
    from jax.sharding import Mesh, NamedSharding, PartitionSpec
    from concourse.bass2jax import (
        _bass_exec_p,
        install_neuronx_cc_hook,
        partition_id_tensor,
    )

    install_neuronx_cc_hook()
    nc = _build(chunks_t, first, last, max(2, overlap))

    partition_name = nc.partition_id_tensor.name if nc.partition_id_tensor else None
    in_names: list[str] = []
    out_names: list[str] = []
    out_avals = []
    for alloc in nc.m.functions[0].allocations:
        if not isinstance(alloc, mybir.MemoryLocationSet):
            continue
        name = alloc.memorylocations[0].name
        if alloc.kind == "ExternalInput":
            if name != partition_name:
                in_names.append(name)
        elif alloc.kind == "ExternalOutput":
            out_names.append(name)
            out_avals.append(
                jax.core.ShapedArray(
                    tuple(alloc.tensor_shape), mybir.dt.np(alloc.dtype)
                )
            )
    n_params = len(in_names)
    n_outs = len(out_names)
    all_names = list(in_names) + list(out_names)
    if partition_name is not None:
        all_names.append(partition_name)
    all_names = tuple(all_names)
    donate = tuple(range(n_params, n_params + n_outs))

    def _body(*args):
        operands = list(args)
        if partition_name is not None:
            operands.append(partition_id_tensor())
        outs = _bass_exec_p.bind(
            *operands,
            out_avals=tuple(out_avals),
            in_names=all_names,
            out_names=tuple(out_names),
            lowering_input_output_aliases=(),
            sim_require_finite=True,
            sim_require_nnan=True,
            nc=nc,
        )
        return tuple(outs)

    devices = jax.devices()[:NCORES]
    mesh = Mesh(np.asarray(devices), ("core",))
    spec = PartitionSpec("core")
    fn = jax.jit(
        shard_map(
            _body,
            mesh=mesh,
            in_specs=(spec,) * (n_params + n_outs),
            out_specs=(spec,) * n_outs,
            check_rep=False,
        ),
        donate_argnums=donate,
        keep_unused=True,
    )
    entry = {
        "fn": fn,
        "in_names": tuple(in_names),
        "out_names": tuple(out_names),
        "sharding": NamedSharding(mesh, spec),
        "nc": nc,
    }
    _CACHE[chunks_t] = entry
    return entry


def _run_fast(x, word_ids, W, b):
    import jax

    wid64, xt, widf, widc, rcc, wtk, bias_bc = _prep_host(x, word_ids, W, b)
    # Globals along axis 0: per-core shard = rows [2*core, 2*core+2); the
    # replicated head weights are tiled 8x (tiny).
    globals_np = {
        "xt": xt,
        "widr": widf,
        "widc": widc,
        "rcc": rcc,
        "wt": np.tile(wtk, (NCORES, 1, 1)),
        "bias": np.tile(bias_bc, (NCORES, 1)),
    }
    zero_out = np.zeros((B, 128, T * C), dtype=np.float32)

    chunks_t, first, last, overlap = _schedule(wid64)
    entry = _CACHE.get(chunks_t)
    if entry is not None:
        # Warm path: start the async upload before dispatch so the transfer
        # overlaps host-side arg marshalling.
        sh = entry["sharding"]
        names = list(globals_np)
        devs = jax.device_put(
            [globals_np[n] for n in names] + [zero_out], [sh] * (len(names) + 1)
        )
        dev_map = dict(zip(names, devs[:-1]))
        out_devs = devs[-1:]
    else:
        entry = _get_compiled(chunks_t, first, last, overlap)
        sh = entry["sharding"]
        names = list(globals_np)
        devs = jax.device_put(
            [globals_np[n] for n in names] + [zero_out], [sh] * (len(names) + 1)
        )
        dev_map = dict(zip(names, devs[:-1]))
        out_devs = devs[-1:]

    args = [dev_map[n] for n in entry["in_names"]] + out_devs
    outs = entry["fn"](*args)
    out = np.asarray(outs[0])  # [B, 128, T*C]
    return (
        np.ascontiguousarray(
            out.reshape(B, 128, T, C).transpose(0, 2, 1, 3).reshape(B, S, C)
        ),
        None,
    )


def _run_fallback(x, word_ids, W, b, **spmd_kwargs):
    from concourse.bass_utils import run_bass_kernel_spmd

    wid64, xt, widf, widc, rcc, wtk, bias_bc = _prep_host(x, word_ids, W, b)
    chunks_t, first, last, overlap = _schedule(wid64)
    nc = _build(chunks_t, first, last, max(2, overlap))
    in_maps = []
    for core in range(NCORES):
        r0 = core * RPC
        in_maps.append(
            {
                "xt": xt[r0 : r0 + RPC],
                "widr": widf[r0 : r0 + RPC],
                "widc": widc[r0 : r0 + RPC],
                "rcc": rcc[r0 : r0 + RPC],
                "wt": wtk,
                "bias": bias_bc,
            }
        )
    res = run_bass_kernel_spmd(nc, in_maps, list(range(NCORES)), **spmd_kwargs)
    outs = []
    for core in range(NCORES):
        o = res.results[core]["out"]  # [RPC, 128, T*C]
        outs.append(o.reshape(RPC, 128, T, C).transpose(0, 2, 1, 3).reshape(RPC, S, C))
    return np.ascontiguousarray(np.concatenate(outs, axis=0).astype(np.float32)), res


def _run(x, word_ids, W, b, **spmd_kwargs):
    if spmd_kwargs.get("trace"):
        return _run_fallback(x, word_ids, W, b, **spmd_kwargs)
    try:
        return _run_fast(x, word_ids, W, b)
    except Exception:
        import traceback

        traceback.print_exc()
        return _run_fallback(x, word_ids, W, b)


def kernel(x, word_ids, W, b):
    return _run(x, word_ids, W, b)[0]


if __name__ == "__main__":
    rng = np.random.default_rng(0)
    x = rng.standard_normal((B, S, H), dtype=np.float32)
    wid = np.sort(rng.integers(0, NW, (B, S)), axis=-1)
    W = rng.standard_normal((C, H), dtype=np.float32) / np.sqrt(H)
    b = np.zeros((C,), dtype=np.float32)
    out = kernel(x, wid, W, b)
    print(out.shape, out.dtype)
